# revision 21
# baseline (speedup 1.0000x reference)
"""Multi-head attention (B=8, S=1024, E=768, H=12, D=64) on 8 TRN2 NeuronCores.

Sharding: data-parallel over batch. Core i computes batch element i end to end;
weights are replicated. No collectives.

Per-core dataflow (all matmuls float32r unless noted; Q/K in bf16):
  1. x [S,E] -> PE-transpose -> xT [E,S] (f32r)
  2. QT/KT = w_qkv.T @ xT (bf16); V -> V_pad [S, H*(D+1)] with a ones column
  3. attention per head pair, per q-half (512 queries), packed K=64 score
     matmuls via tile_position; exp on ACT in [128,512] chunks so four
     1-bank PSUM score slots rotate chain-free; PV accumulates out^T[d,q]
     plus a denominator row; normalize = PE broadcast + fast reciprocal
  4. y = attnT.T @ w_out + b_out

PSUM budget (8 banks): scores 4 x 1 + pv 2 x 1 + mm 2 x 1.
"""

import numpy as np

import concourse.bass as bass
import concourse.bacc as bacc
import concourse.tile as tile
from concourse import mybir
from concourse.bass_utils import run_bass_kernel_spmd
from concourse.bass_interp import get_hw_module
from concourse.masks import make_identity

F32 = mybir.dt.float32
F32R = mybir.dt.float32r
BF16 = mybir.dt.bfloat16
U32 = mybir.dt.uint32

B, S, E = 8, 1024, 768
H, D = 12, 64
F = 3 * E                  # 2304
NCORES = 8
NPAIR = H // 2             # 6 head pairs
NKC = S // 128             # 8 key chunks
NST = S // 128             # 8 sequence tiles
NE = E // 128              # 6 embedding chunks
DP = D + 1                 # 65: head dim + ones column

PV_DT = F32R               # dtype of exp(S^T) and V_pad
QK_DT = BF16               # dtype of Q^T / K^T


def _build():
    nc = bacc.Bacc("TRN2", target_bir_lowering=False, debug=False,
                   num_devices=NCORES)

    x_d = nc.dram_tensor("x", [S, E], F32, kind="ExternalInput").ap()
    wqkv_d = nc.dram_tensor("w_qkv", [E, F], F32, kind="ExternalInput").ap()
    wout_d = nc.dram_tensor("w_out", [E, E], F32, kind="ExternalInput").ap()
    bout_d = nc.dram_tensor("b_out", [E], F32, kind="ExternalInput").ap()
    y_d = nc.dram_tensor("y", [S, E], F32, kind="ExternalOutput").ap()

    with tile.TileContext(nc) as tc:
        _emit(nc, tc, x_d, wqkv_d, wout_d, bout_d, y_d)

    nc.compile()
    nc.m = get_hw_module(nc.m)
    return nc


def _emit(nc, tc, x_d, wqkv_d, wout_d, bout_d, y_d):
    from contextlib import ExitStack
    ctx = ExitStack()
    with ctx:
        singles = ctx.enter_context(tc.tile_pool(name="singles", bufs=1))
        sb = ctx.enter_context(tc.tile_pool(name="sb", bufs=1))
        ps = ctx.enter_context(tc.tile_pool(name="ps", bufs=1, space="PSUM"))

        # ---- constants ----
        identity = singles.tile([128, 128], F32)
        make_identity(nc, identity)
        bias_bc = singles.tile([128, E], F32)
        nc.sync.dma_start(
            out=bias_bc,
            in_=bass.AP(tensor=bout_d.tensor, offset=bout_d.offset,
                        ap=[[0, 128]] + list(bout_d.ap)))
        ones_row = singles.tile([1, 64], F32R)
        nc.gpsimd.memset(ones_row.bitcast(U32), 0x3F800000)

        qkv_pool = tc.alloc_tile_pool(name="qkv_pool", bufs=1)

        # ---- weights + x ----
        wq = []
        for ei in range(NE):
            w_t = qkv_pool.tile([128, F], F32R, name=f"wqkv_{ei}")
            nc.sync.dma_start(out=w_t,
                              in_=wqkv_d[ei * 128:(ei + 1) * 128, :].bitcast(F32R))
            wq.append(w_t)
        x_sb = []
        for st in range(NST):
            x_t = qkv_pool.tile([128, E], F32, name=f"x_{st}")
            nc.sync.dma_start(out=x_t, in_=x_d[st * 128:(st + 1) * 128, :])
            x_sb.append(x_t)

        # ---- transpose x -> xT [E, S] (f32r) via PE, 1-bank psum chunks ----
        xT = []
        for ei in range(NE):
            xt_t = qkv_pool.tile([128, S], F32R, name=f"xT_{ei}")
            for half in range(2):
                ps_xt = ps.tile([128, 512], F32, tag="mm", bufs=2,
                                name=f"psxt_{ei}_{half}")
                for k in range(4):
                    st = half * 4 + k
                    nc.tensor.transpose(
                        ps_xt[:, k * 128:(k + 1) * 128],
                        x_sb[st][:, ei * 128:(ei + 1) * 128],
                        identity)
                nc.vector.tensor_copy(xt_t[:, half * 512:(half + 1) * 512],
                                      ps_xt)
            xT.append(xt_t)

        # ---- V projection -> V_pad [S, H*DP] (ones col per head) ----
        v_pad = []
        for st in range(NST):
            vp_t = sb.tile([128, H * DP], PV_DT, name=f"vpad_{st}")
            vp3 = vp_t.rearrange("p (h c) -> p h c", c=DP)
            nc.gpsimd.memset(vp_t.bitcast(U32), 0x3F800000)
            for (c0, cn) in ((0, 512), (512, 256)):
                ps_v = ps.tile([128, 512], F32, tag="mm", bufs=2,
                               name=f"psv_{st}_{c0}")
                for ei in range(NE):
                    nc.tensor.matmul(
                        ps_v[:, 0:cn],
                        xT[ei][:, st * 128:(st + 1) * 128],
                        wq[ei][:, 2 * E + c0:2 * E + c0 + cn],
                        start=(ei == 0), stop=(ei == NE - 1))
                # scatter into per-head 65-wide blocks
                h0 = c0 // D
                nh = cn // D
                nc.vector.tensor_copy(
                    vp3[:, h0:h0 + nh, 0:D],
                    ps_v[:, 0:cn].rearrange("p (h d) -> p h d", d=D))
            v_pad.append(vp_t)

        # ---- Q^T, K^T projections (bf16) ----
        qkT = []          # 0-5 = Q^T pair tiles, 6-11 = K^T pair tiles
        for ft in range(2 * NE):
            qk_t = sb.tile([128, S], QK_DT, name=f"qkT_{ft}")
            for sc in range(2):
                ps_q = ps.tile([128, 512], F32, tag="mm", bufs=2,
                               name=f"psq_{ft}_{sc}")
                for ei in range(NE):
                    nc.tensor.matmul(
                        ps_q,
                        wq[ei][:, ft * 128:(ft + 1) * 128],
                        xT[ei][:, sc * 512:(sc + 1) * 512],
                        start=(ei == 0), stop=(ei == NE - 1))
                nc.vector.tensor_copy(qk_t[:, sc * 512:(sc + 1) * 512], ps_q)
            qkT.append(qk_t)

        qkv_pool.release()
        expst_pool = ctx.enter_context(tc.tile_pool(name="expst", bufs=4))
        bcast_pool = ctx.enter_context(tc.tile_pool(name="bcast", bufs=2))
        rc_pool = ctx.enter_context(tc.tile_pool(name="rc", bufs=2))
        wpool = ctx.enter_context(tc.tile_pool(name="wpool", bufs=1))
        ypool = ctx.enter_context(tc.tile_pool(name="ypool", bufs=2))

        wo = []
        for ei in range(NE):
            wo_t = wpool.tile([128, E], F32R, name=f"wout_{ei}")
            nc.sync.dma_start(out=wo_t,
                              in_=wout_d[ei * 128:(ei + 1) * 128, :].bitcast(F32R))
            wo.append(wo_t)

        # ---- attention: per pair, per q-half ----
        attnT = []
        for j in range(NPAIR):
            qT = qkT[j]
            kT = qkT[NE + j]
            at_t = sb.tile([128, S], F32R, name=f"attnT_{j}")
            for qh in range(2):
                q0 = qh * 512
                ps_pv = [ps.tile([DP, 512], F32, tag="pv", bufs=2,
                                 name=f"pspv_{j}_{qh}_{hh}")
                         for hh in range(2)]
                expst_tiles = {}

                def emit_pv(kc, j=j, qh=qh, ps_pv=ps_pv,
                            expst_tiles=expst_tiles):
                    expst = expst_tiles.pop(kc)
                    for hh in range(2):
                        nc.tensor.matmul(
                            ps_pv[hh],
                            v_pad[kc][:, (2 * j + hh) * DP:
                                       (2 * j + hh + 1) * DP],
                            expst[:, hh * 512:(hh + 1) * 512],
                            start=(kc == 0), stop=(kc == NKC - 1))

                for kc in range(NKC):
                    expst = expst_pool.tile([128, 1024], PV_DT, tag="expst",
                                            name=f"expst_{j}_{qh}_{kc}")
                    expst_tiles[kc] = expst
                    for hh in range(2):
                        ps_s = ps.tile([128, 512], F32, tag="scores", bufs=4,
                                       name=f"pss_{j}_{qh}_{kc}_{hh}")
                        nc.tensor.matmul(
                            ps_s,
                            kT[hh * 64:(hh + 1) * 64,
                               kc * 128:(kc + 1) * 128],
                            qT[hh * 64:(hh + 1) * 64, q0:q0 + 512],
                            start=True, stop=True,
                            tile_position=(hh * 64, 0))
                        nc.scalar.activation(
                            out=expst[:, hh * 512:(hh + 1) * 512], in_=ps_s,
                            func=mybir.ActivationFunctionType.Exp,
                            scale=0.125)
                    if kc > 0:
                        emit_pv(kc - 1)
                emit_pv(NKC - 1)

                # normalize: denom row -> PE broadcast -> approx recip -> mul
                rc_t = rc_pool.tile([1, 1024], F32R, tag="rc",
                                    name=f"rc_{j}_{qh}")
                for hh in range(2):
                    nc.vector.tensor_copy(rc_t[0:1, hh * 512:(hh + 1) * 512],
                                          ps_pv[hh][D:DP, :])
                bc_sb = bcast_pool.tile([64, 1024], F32, tag="bc",
                                        name=f"bc_{j}_{qh}")
                for hh in range(2):
                    bc_ps = ps.tile([64, 512], F32, tag="scores", bufs=4,
                                    name=f"bcps_{j}_{qh}_{hh}")
                    nc.tensor.matmul(bc_ps, ones_row,
                                     rc_t[0:1, hh * 512:(hh + 1) * 512],
                                     start=True, stop=True)
                    nc.vector.reciprocal_approx_fast(
                        out=bc_sb[:, hh * 512:(hh + 1) * 512], in_=bc_ps)
                for hh in range(2):
                    nc.vector.tensor_mul(
                        at_t[hh * 64:(hh + 1) * 64, q0:q0 + 512],
                        ps_pv[hh][0:D, :],
                        bc_sb[:, hh * 512:(hh + 1) * 512])
            attnT.append(at_t)

        # ---- output projection + bias ----
        for st in range(NST):
            y_t = ypool.tile([128, E], F32, tag="y", name=f"y_{st}")
            for (c0, cn) in ((0, 512), (512, 256)):
                ps_y = ps.tile([128, 512], F32, tag="mm", bufs=2,
                               name=f"psy_{st}_{c0}")
                for ej in range(NE):
                    nc.tensor.matmul(
                        ps_y[:, 0:cn],
                        attnT[ej][:, st * 128:(st + 1) * 128],
                        wo[ej][:, c0:c0 + cn],
                        start=(ej == 0), stop=(ej == NE - 1))
                nc.vector.tensor_add(y_t[:, c0:c0 + cn], ps_y[:, 0:cn],
                                     bias_bc[:, c0:c0 + cn])
            nc.sync.dma_start(out=y_d[st * 128:(st + 1) * 128, :], in_=y_t)


_NC_CACHE = None


def _get_nc():
    global _NC_CACHE
    if _NC_CACHE is None:
        _NC_CACHE = _build()
    return _NC_CACHE


def kernel(x, w_qkv, w_out, b_out, _trace=False, **_run_kwargs):
    """Full-input MHA: x [8,1024,768] f32 -> y [8,1024,768] f32."""
    nc = _get_nc()
    x = np.ascontiguousarray(np.asarray(x, dtype=np.float32))
    w_qkv = np.ascontiguousarray(np.asarray(w_qkv, dtype=np.float32))
    w_out = np.ascontiguousarray(np.asarray(w_out, dtype=np.float32))
    b_out = np.ascontiguousarray(np.asarray(b_out, dtype=np.float32))
    in_maps = [
        {"x": x[i], "w_qkv": w_qkv, "w_out": w_out, "b_out": b_out}
        for i in range(NCORES)
    ]
    res = run_bass_kernel_spmd(nc, in_maps, core_ids=list(range(NCORES)),
                               trace=_trace, **_run_kwargs)
    y = np.stack([res.results[i]["y"] for i in range(NCORES)], axis=0)
    if _trace:
        return y, res
    return y


# revision 28
# speedup vs baseline: 1.0970x; 1.0970x over previous
"""Multi-head attention (B=8, S=1024, E=768, H=12, D=64) on 8 TRN2 NeuronCores.

Sharding: data-parallel over batch. Core i computes batch element i end to end;
weights are replicated. No collectives.

Per-core dataflow (all matmuls float32r unless noted; Q/K in bf16):
  1. x [S,E] -> PE-transpose -> xT [E,S] (f32r)
  2. QT/KT = w_qkv.T @ xT (bf16); V -> V_pad [S, H*(D+1)] with a ones column
  3. attention per head pair, per q-half (512 queries), packed K=64 score
     matmuls via tile_position; exp on ACT in [128,512] chunks over four
     1-bank PSUM score slots; PV accumulates out^T[d,q] plus a denominator
     row; normalize = PE broadcast + fast reciprocal
  4. y = attnT.T @ w_out + b_out

The V and Q/K projection work is interleaved into the attention loop as PE
"filler" chunks: attention alone leaves the PE ~55% idle waiting on ACT exp,
which also parks the PE clock at 1.2 GHz (HAM never sees a dense-activity
window). Interleaving keeps the PE dense and the clock at 2.4 GHz.

PSUM budget (8 banks): scores 4 x 1 + pv 2 x 1 + mm 2 x 1.
"""

import numpy as np

import concourse.bass as bass
import concourse.bacc as bacc
import concourse.tile as tile
from concourse import mybir
from concourse.bass_utils import run_bass_kernel_spmd
from concourse.bass_interp import get_hw_module
from concourse.masks import make_identity

F32 = mybir.dt.float32
F32R = mybir.dt.float32r
BF16 = mybir.dt.bfloat16
U32 = mybir.dt.uint32

B, S, E = 8, 1024, 768
H, D = 12, 64
F = 3 * E                  # 2304
NCORES = 8
NPAIR = H // 2             # 6 head pairs
NKC = S // 128             # 8 key chunks
NST = S // 128             # 8 sequence tiles
NE = E // 128              # 6 embedding chunks
DP = D + 1                 # 65: head dim + ones column

PV_DT = F32R               # dtype of exp(S^T) and V_pad
QK_DT = BF16               # dtype of Q^T / K^T


def _build():
    nc = bacc.Bacc("TRN2", target_bir_lowering=False, debug=False,
                   num_devices=NCORES)

    x_d = nc.dram_tensor("x", [S, E], F32, kind="ExternalInput").ap()
    wqkv_d = nc.dram_tensor("w_qkv", [E, F], F32, kind="ExternalInput").ap()
    wout_d = nc.dram_tensor("w_out", [E, E], F32, kind="ExternalInput").ap()
    bout_d = nc.dram_tensor("b_out", [E], F32, kind="ExternalInput").ap()
    y_d = nc.dram_tensor("y", [S, E], F32, kind="ExternalOutput").ap()

    with tile.TileContext(nc) as tc:
        _emit(nc, tc, x_d, wqkv_d, wout_d, bout_d, y_d)

    nc.compile()
    nc.m = get_hw_module(nc.m)
    return nc


def _emit(nc, tc, x_d, wqkv_d, wout_d, bout_d, y_d):
    from contextlib import ExitStack
    ctx = ExitStack()
    with ctx:
        singles = ctx.enter_context(tc.tile_pool(name="singles", bufs=1))
        sb = ctx.enter_context(tc.tile_pool(name="sb", bufs=1))
        ps = ctx.enter_context(tc.tile_pool(name="ps", bufs=1, space="PSUM"))
        expst_pool = ctx.enter_context(tc.tile_pool(name="expst", bufs=3))
        bcast_pool = ctx.enter_context(tc.tile_pool(name="bcast", bufs=2))
        rc_pool = ctx.enter_context(tc.tile_pool(name="rc", bufs=2))
        ypool = ctx.enter_context(tc.tile_pool(name="ypool", bufs=2))

        # ---- constants ----
        identity = singles.tile([128, 128], F32)
        make_identity(nc, identity)
        bias_bc = singles.tile([128, E], F32)
        nc.sync.dma_start(
            out=bias_bc,
            in_=bass.AP(tensor=bout_d.tensor, offset=bout_d.offset,
                        ap=[[0, 128]] + list(bout_d.ap)))
        ones_row = singles.tile([1, 64], F32R)
        nc.gpsimd.memset(ones_row.bitcast(U32), 0x3F800000)

        wq_pool = tc.alloc_tile_pool(name="wq_pool", bufs=1)
        x_pool = tc.alloc_tile_pool(name="x_pool", bufs=1)

        # ---- weights + x ----
        wq = []
        for ei in range(NE):
            w_t = wq_pool.tile([128, F], F32R, name=f"wqkv_{ei}")
            nc.sync.dma_start(out=w_t,
                              in_=wqkv_d[ei * 128:(ei + 1) * 128, :].bitcast(F32R))
            wq.append(w_t)
        # ---- transpose x -> xT [E, S] (f32r) via PE, half of x at a time ----
        xT = [wq_pool.tile([128, S], F32R, name=f"xT_{ei}")
              for ei in range(NE)]
        for half in range(2):
            x_sb = []
            for k in range(4):
                st = half * 4 + k
                x_t = x_pool.tile([128, E], F32, tag="x", bufs=4,
                                  name=f"x_{st}")
                nc.sync.dma_start(out=x_t, in_=x_d[st * 128:(st + 1) * 128, :])
                x_sb.append(x_t)
            for ei in range(NE):
                ps_xt = ps.tile([128, 512], F32, tag="mm", bufs=2,
                                name=f"psxt_{ei}_{half}")
                for k in range(4):
                    nc.tensor.transpose(
                        ps_xt[:, k * 128:(k + 1) * 128],
                        x_sb[k][:, ei * 128:(ei + 1) * 128],
                        identity)
                nc.vector.tensor_copy(xT[ei][:, half * 512:(half + 1) * 512],
                                      ps_xt)
        x_pool.release()

        # ---- projection chunk emitters (used as PE fillers) ----
        v_pad = [sb.tile([128, H * DP], PV_DT, name=f"vpad_{st}")
                 for st in range(NST)]
        for st in range(NST):
            nc.gpsimd.memset(v_pad[st].bitcast(U32), 0x3F800000)
        qkT = [sb.tile([128, S], QK_DT, name=f"qkT_{ft}")
               for ft in range(2 * NE)]

        def emit_v_chunk(st, c0, cn):
            ps_v = ps.tile([128, 512], F32, tag="mm", bufs=2,
                           name=f"psv_{st}_{c0}")
            for ei in range(NE):
                nc.tensor.matmul(
                    ps_v[:, 0:cn],
                    xT[ei][:, st * 128:(st + 1) * 128],
                    wq[ei][:, 2 * E + c0:2 * E + c0 + cn],
                    start=(ei == 0), stop=(ei == NE - 1))
            vp3 = v_pad[st].rearrange("p (h c) -> p h c", c=DP)
            h0 = c0 // D
            nc.vector.tensor_copy(
                vp3[:, h0:h0 + cn // D, 0:D],
                ps_v[:, 0:cn].rearrange("p (h d) -> p h d", d=D))

        def emit_qkt_chunk(ft, sc):
            ps_q = ps.tile([128, 512], F32, tag="mm", bufs=2,
                           name=f"psq_{ft}_{sc}")
            for ei in range(NE):
                nc.tensor.matmul(
                    ps_q,
                    wq[ei][:, ft * 128:(ft + 1) * 128],
                    xT[ei][:, sc * 512:(sc + 1) * 512],
                    start=(ei == 0), stop=(ei == NE - 1))
            nc.vector.tensor_copy(qkT[ft][:, sc * 512:(sc + 1) * 512], ps_q)

        # prelude: V st=0 and Q/K for pair 0 must exist before attention
        for sc in range(2):
            emit_qkt_chunk(0, sc)
            emit_qkt_chunk(NE, sc)
        emit_v_chunk(0, 0, 512)
        emit_v_chunk(0, 512, 256)

        # filler schedule: (j, qh, kc) -> list of emit thunks.
        # V(st) chunks land in pair0/qh0 two-per-iteration so v_pad[kc] is
        # ready ~2 iterations before PV(kc) consumes it; QKT for pair j+1 is
        # spread across pair j so every pair has some PE filler and the
        # weights are ready one pair ahead.
        filler_schedule = {}

        def sched(j, qh, kc, thunk):
            filler_schedule.setdefault((j, qh, kc), []).append(thunk)

        for st in range(1, NST):
            kc = st - 1
            sched(0, 0, kc, lambda st=st: emit_v_chunk(st, 0, 512))
            sched(0, 0, kc, lambda st=st: emit_v_chunk(st, 512, 256))
        for j in range(1, NPAIR):
            for i, (ft, sc) in enumerate(
                    [(j, 0), (NE + j, 0), (j, 1), (NE + j, 1)]):
                sched(j - 1, i // 2, 1 + 3 * (i % 2),
                      lambda ft=ft, sc=sc: emit_qkt_chunk(ft, sc))

        def pop_filler(j, qh, kc):
            for thunk in filler_schedule.pop((j, qh, kc), ()):
                thunk()

        # ---- attention: per pair, per q-half, fillers interleaved ----
        attnT = []
        for j in range(NPAIR):
            qT = qkT[j]
            kT = qkT[NE + j]
            at_t = sb.tile([128, S], F32R, name=f"attnT_{j}")
            for qh in range(2):
                q0 = qh * 512
                ps_pv = [ps.tile([DP, 512], F32, tag="pv", bufs=2,
                                 name=f"pspv_{j}_{qh}_{hh}")
                         for hh in range(2)]
                expst_tiles = {}

                def emit_pv(kc, j=j, qh=qh, ps_pv=ps_pv,
                            expst_tiles=expst_tiles):
                    expst = expst_tiles.pop(kc)
                    for hh in range(2):
                        nc.tensor.matmul(
                            ps_pv[hh],
                            v_pad[kc][:, (2 * j + hh) * DP:
                                       (2 * j + hh + 1) * DP],
                            expst[:, hh * 512:(hh + 1) * 512],
                            start=(kc == 0), stop=(kc == NKC - 1))

                for kc in range(NKC):
                    expst = expst_pool.tile([128, 1024], PV_DT, tag="expst",
                                            name=f"expst_{j}_{qh}_{kc}")
                    expst_tiles[kc] = expst
                    for hh in range(2):
                        ps_s = ps.tile([128, 512], F32, tag="scores", bufs=4,
                                       name=f"pss_{j}_{qh}_{kc}_{hh}")
                        nc.tensor.matmul(
                            ps_s,
                            kT[hh * 64:(hh + 1) * 64,
                               kc * 128:(kc + 1) * 128],
                            qT[hh * 64:(hh + 1) * 64, q0:q0 + 512],
                            start=True, stop=True,
                            tile_position=(hh * 64, 0))
                        nc.scalar.activation(
                            out=expst[:, hh * 512:(hh + 1) * 512], in_=ps_s,
                            func=mybir.ActivationFunctionType.Exp,
                            scale=0.125)
                    if kc > 0:
                        emit_pv(kc - 1)
                    pop_filler(j, qh, kc)
                emit_pv(NKC - 1)

                # normalize: denom row -> PE broadcast -> approx recip -> mul
                bc_sb = bcast_pool.tile([64, 1024], F32, tag="bc",
                                        name=f"bc_{j}_{qh}")
                for hh in range(2):
                    rc_t = rc_pool.tile([1, 512], F32R, tag="rc",
                                        name=f"rc_{j}_{qh}_{hh}")
                    nc.vector.tensor_copy(rc_t, ps_pv[hh][D:DP, :])
                    bc_ps = ps.tile([64, 512], F32, tag="scores", bufs=4,
                                    name=f"bcps_{j}_{qh}_{hh}")
                    nc.tensor.matmul(bc_ps, ones_row, rc_t,
                                     start=True, stop=True)
                    nc.vector.reciprocal_approx_fast(
                        out=bc_sb[:, hh * 512:(hh + 1) * 512], in_=bc_ps)
                for hh in range(2):
                    nc.vector.tensor_mul(
                        at_t[hh * 64:(hh + 1) * 64, q0:q0 + 512],
                        ps_pv[hh][0:D, :],
                        bc_sb[:, hh * 512:(hh + 1) * 512])
            attnT.append(at_t)
        for key in sorted(filler_schedule):
            for thunk in filler_schedule[key]:
                thunk()
        filler_schedule.clear()
        wq_pool.release()

        # ---- output projection + bias ----
        wpool = ctx.enter_context(tc.tile_pool(name="wpool", bufs=1))
        wo = []
        for ei in range(NE):
            wo_t = wpool.tile([128, E], F32R, name=f"wout_{ei}")
            nc.sync.dma_start(out=wo_t,
                              in_=wout_d[ei * 128:(ei + 1) * 128, :].bitcast(F32R))
            wo.append(wo_t)

        for st in range(NST):
            y_t = ypool.tile([128, E], F32, tag="y", name=f"y_{st}")
            for (c0, cn) in ((0, 512), (512, 256)):
                ps_y = ps.tile([128, 512], F32, tag="mm", bufs=2,
                               name=f"psy_{st}_{c0}")
                for ej in range(NE):
                    nc.tensor.matmul(
                        ps_y[:, 0:cn],
                        attnT[ej][:, st * 128:(st + 1) * 128],
                        wo[ej][:, c0:c0 + cn],
                        start=(ej == 0), stop=(ej == NE - 1))
                nc.vector.tensor_add(y_t[:, c0:c0 + cn], ps_y[:, 0:cn],
                                     bias_bc[:, c0:c0 + cn])
            nc.sync.dma_start(out=y_d[st * 128:(st + 1) * 128, :], in_=y_t)


_NC_CACHE = None


def _get_nc():
    global _NC_CACHE
    if _NC_CACHE is None:
        _NC_CACHE = _build()
    return _NC_CACHE


def kernel(x, w_qkv, w_out, b_out, _trace=False, **_run_kwargs):
    """Full-input MHA: x [8,1024,768] f32 -> y [8,1024,768] f32."""
    nc = _get_nc()
    x = np.ascontiguousarray(np.asarray(x, dtype=np.float32))
    w_qkv = np.ascontiguousarray(np.asarray(w_qkv, dtype=np.float32))
    w_out = np.ascontiguousarray(np.asarray(w_out, dtype=np.float32))
    b_out = np.ascontiguousarray(np.asarray(b_out, dtype=np.float32))
    in_maps = [
        {"x": x[i], "w_qkv": w_qkv, "w_out": w_out, "b_out": b_out}
        for i in range(NCORES)
    ]
    res = run_bass_kernel_spmd(nc, in_maps, core_ids=list(range(NCORES)),
                               trace=_trace, **_run_kwargs)
    y = np.stack([res.results[i]["y"] for i in range(NCORES)], axis=0)
    if _trace:
        return y, res
    return y
